# revision 1
# baseline (speedup 1.0000x reference)
"""GraphSAGE-style 2-layer GNN minibatch forward on 8 trn2 NeuronCores.

Data-parallel over the 1024 target nodes: each core handles 128 targets.
Per core: one dma_gather per 128-token group fetches all 26 feature rows
per token (self + 25 neighbors, 3328 rows / 3.3 MB per group) from a
host-deduplicated per-core feature subtable (int16 local ids), then a
strided DVE reduce forms the neighbor sum, PE transposes + fp32 matmuls
apply the MLP, and ACT/DVE do relu + l2-normalization.

All shapes hardcoded; self-contained (only needs the concourse runtime
that ships with the container).
"""

import numpy as np

N_CORES = 8
N_NODES = 100000
D = 256          # feature dim
P = 128          # partitions / tokens per group
B = 1024         # total targets
S0 = 25          # layer-0 fanout
S1 = 10          # layer-1 fanout
NG = 11          # groups of 128 tokens per core at layer 1 (1408 = 11*128)
NSLOT = 1 + S0   # gather slots per token (self + neighbors)
VLOC = 32768     # per-core deduplicated subtable size (int16 index space)
NIDX = P * NSLOT          # 3328 rows per group gather
NCOLS = NIDX // 16        # idx tile free dim (16-partition wrap layout)

_CACHE = {}


def _build_program():
    import concourse.bacc as bacc
    import concourse.mybir as mybir
    import concourse.tile as tile
    from concourse.masks import make_identity

    F32 = mybir.dt.float32
    I16 = mybir.dt.int16
    AF = mybir.ActivationFunctionType
    ALU = mybir.AluOpType
    AX = mybir.AxisListType

    nc = bacc.Bacc("TRN2", target_bir_lowering=False, debug=False)

    feats = nc.dram_tensor("feats", [VLOC, D], F32, kind="ExternalInput")
    idx16_d = nc.dram_tensor("idx16", [NG * P, NCOLS], I16, kind="ExternalInput")
    w0t_d = nc.dram_tensor("w0t", [2 * D, D], F32, kind="ExternalInput")
    w1t_d = nc.dram_tensor("w1t", [2 * D, D], F32, kind="ExternalInput")
    b0_d = nc.dram_tensor("b0", [1, D], F32, kind="ExternalInput")
    b1_d = nc.dram_tensor("b1", [1, D], F32, kind="ExternalInput")
    a1_d = nc.dram_tensor("a1", [S1, P, P], F32, kind="ExternalInput")
    out_d = nc.dram_tensor("out", [P, D], F32, kind="ExternalOutput")

    with tile.TileContext(nc) as tc:
        with (
            tc.tile_pool(name="consts", bufs=1) as consts,
            tc.tile_pool(name="idxp", bufs=3) as idxp,
            tc.tile_pool(name="gatp", bufs=3) as gatp,
            tc.tile_pool(name="aggp", bufs=2) as aggp,
            tc.tile_pool(name="xtp", bufs=2) as xtp,
            tc.tile_pool(name="epip", bufs=2) as epip,
            tc.tile_pool(name="tpp", bufs=4, space="PSUM") as tpp,
            tc.tile_pool(name="mmp", bufs=2, space="PSUM") as mmp,
        ):
            ident = consts.tile([P, P], F32, tag="ident")
            make_identity(nc, ident[:])
            ones1 = consts.tile([1, P], F32, tag="ones1")
            nc.vector.memset(ones1[:], 1.0)
            eps = consts.tile([P, 1], F32, tag="eps")
            nc.vector.memset(eps[:], 1e-30)

            w0t_sb, w1t_sb = [], []
            for kc in range(4):
                t0 = consts.tile([P, D], F32, tag=f"w0t{kc}")
                nc.sync.dma_start(out=t0[:], in_=w0t_d[kc * P:(kc + 1) * P, :])
                w0t_sb.append(t0)
                t1 = consts.tile([P, D], F32, tag=f"w1t{kc}")
                nc.sync.dma_start(out=t1[:], in_=w1t_d[kc * P:(kc + 1) * P, :])
                w1t_sb.append(t1)
            b0_sb = consts.tile([1, D], F32, tag="b0")
            nc.sync.dma_start(out=b0_sb[:], in_=b0_d[:])
            b1_sb = consts.tile([1, D], F32, tag="b1")
            nc.sync.dma_start(out=b1_sb[:], in_=b1_d[:])
            a1_sb = []
            for j in range(S1):
                t = consts.tile([P, P], F32, tag=f"a1_{j}")
                nc.sync.dma_start(out=t[:], in_=a1_d[j])
                a1_sb.append(t)
            h1_sb = [
                consts.tile([P, D], F32, tag=f"h1_{g}", name=f"h1_{g}")
                for g in range(NG)
            ]
            out_sb = consts.tile([P, D], F32, tag="out_sb")

            def mlp(self_ap, agg_ap, w_sb, b_sb, out_t):
                # out_t = l2norm(relu([self | agg] @ W.T + b)) for 128 tokens.
                xt = []
                for i, (src, col) in enumerate(
                    [(self_ap, 0), (self_ap, 1), (agg_ap, 0), (agg_ap, 1)]
                ):
                    tp = tpp.tile([P, P], F32, tag="tp")
                    nc.tensor.transpose(
                        out=tp[:], in_=src[:, col * P:(col + 1) * P],
                        identity=ident[:],
                    )
                    x = xtp.tile([P, P], F32, tag=f"xt{i}")
                    if i % 2 == 0:
                        nc.vector.tensor_copy(out=x[:], in_=tp[:])
                    else:
                        nc.scalar.copy(out=x[:], in_=tp[:])
                    xt.append(x)
                ph = mmp.tile([P, D], F32, tag="ph")
                for i in range(4):
                    nc.tensor.matmul(
                        out=ph[:], lhsT=xt[i][:], rhs=w_sb[i][:],
                        start=(i == 0), stop=False,
                    )
                # rank-1 bias add: ones[1,P].T @ b[1,D]
                nc.tensor.matmul(
                    out=ph[:], lhsT=ones1[:], rhs=b_sb[:], start=False, stop=True
                )
                h1r = epip.tile([P, D], F32, tag="h1r")
                nc.scalar.activation(out=h1r[:], in_=ph[:], func=AF.Relu)
                trash = epip.tile([P, D], F32, tag="trash")
                n2 = epip.tile([P, 1], F32, tag="n2")
                nc.scalar.activation(
                    out=trash[:], in_=h1r[:], func=AF.Square, accum_out=n2[:]
                )
                nrm = epip.tile([P, 1], F32, tag="nrm")
                nc.scalar.activation(out=nrm[:], in_=n2[:], func=AF.Sqrt, bias=eps[:])
                rinv = epip.tile([P, 1], F32, tag="rinv")
                nc.vector.reciprocal(out=rinv[:], in_=nrm[:])
                # h1r >= 0 and rinv > 0, so relu(h1r * rinv) == h1r * rinv
                nc.scalar.activation(
                    out=out_t[:], in_=h1r[:], func=AF.Relu, scale=rinv[:]
                )

            # ---- layer 0: 11 groups of 128 tokens ----
            for g in range(NG):
                idxt = idxp.tile([P, NCOLS], I16, tag="idxt")
                nc.sync.dma_start(out=idxt[:], in_=idx16_d[g * P:(g + 1) * P, :])
                gat = gatp.tile([P, NSLOT * D], F32, tag="gat")
                nc.gpsimd.dma_gather(
                    out_ap=gat[:].rearrange("p (s d) -> p s d", s=NSLOT),
                    in_ap=feats[:],
                    idxs_ap=idxt[:],
                    num_idxs=NIDX,
                    num_idxs_reg=NIDX,
                    elem_size=D,
                    single_packet=False,
                )
                agg = aggp.tile([P, D], F32, tag="agg")
                nc.vector.tensor_reduce(
                    out=agg[:],
                    in_=gat[:, D:].rearrange("p (s d) -> p d s", s=S0),
                    axis=AX.X, op=ALU.add,
                )
                mlp(gat, agg, w0t_sb, b0_sb, h1_sb[g])

            # ---- layer 1 ----
            pagg = mmp.tile([P, D], F32, tag="ph")
            for j in range(S1):
                nc.tensor.matmul(
                    out=pagg[:], lhsT=a1_sb[j][:], rhs=h1_sb[1 + j][:],
                    start=(j == 0), stop=(j == S1 - 1),
                )
            agg1 = aggp.tile([P, D], F32, tag="agg")
            nc.vector.tensor_copy(out=agg1[:], in_=pagg[:])
            mlp(h1_sb[0], agg1, w1t_sb, b1_sb, out_sb)
            nc.sync.dma_start(out=out_d[:], in_=out_sb[:])

    nc.compile()
    return nc


def get_program():
    if "nc" not in _CACHE:
        _CACHE["nc"] = _build_program()
    return _CACHE["nc"]


def prepare_in_maps(features, W0, b0, W1, b1, nodes2, neigh2, neigh1):
    """Host-side sharding, dedup + int16 remap, and constant prep."""
    features = np.ascontiguousarray(features, dtype=np.float32)
    w0t = np.ascontiguousarray(W0.T, dtype=np.float32).copy()
    w0t[D:, :] /= S0  # fold the layer-0 neighbor mean into the weights
    w1t = np.ascontiguousarray(W1.T, dtype=np.float32).copy()
    w1t[D:, :] /= S1
    b0r = np.ascontiguousarray(b0.reshape(1, D), dtype=np.float32)
    b1r = np.ascontiguousarray(b1.reshape(1, D), dtype=np.float32)

    # layer-1 aggregation matrices: token 128*g + p (g>=1) is neighbor
    # j = 128*(g-1) + p of target j // 10
    a1 = np.zeros((S1, P, P), dtype=np.float32)
    j = np.arange(P * S1)
    a1[j // P, j % P, j // S1] = 1.0

    in_maps = []
    bc = B // N_CORES  # 128 targets per core
    for c in range(N_CORES):
        nodes2_c = nodes2[c * bc:(c + 1) * bc]
        neigh2_c = neigh2[c * bc:(c + 1) * bc, :]
        nodes1_c = np.concatenate([nodes2_c, neigh2_c.reshape(-1)])
        neigh1_c = np.concatenate(
            [
                neigh1[c * bc:(c + 1) * bc, :],
                neigh1[B + c * bc * S1:B + (c + 1) * bc * S1, :],
            ],
            axis=0,
        )
        idx0_c = np.concatenate([nodes1_c[:, None], neigh1_c], axis=1)  # [1408, 26]
        uniq, inv = np.unique(idx0_c.reshape(-1), return_inverse=True)
        assert len(uniq) <= VLOC, f"core {c}: {len(uniq)} unique rows > {VLOC}"
        feats_c = np.zeros((VLOC, D), np.float32)
        feats_c[: len(uniq)] = features[uniq]
        inv = inv.reshape(NG * P, NSLOT).astype(np.int16)  # local ids < 32768

        # per-group slot-major flat order, then 16-partition wrap layout
        idx16 = np.empty((NG * P, NCOLS), np.int16)
        for g in range(NG):
            flat = inv[g * P:(g + 1) * P, :].T.reshape(-1)  # [3328], i = s*128 + p
            idx16[g * P:(g + 1) * P, :] = np.tile(flat.reshape(NCOLS, 16).T, (8, 1))

        in_maps.append(
            {
                "feats": feats_c,
                "idx16": idx16,
                "w0t": w0t,
                "w1t": w1t,
                "b0": b0r,
                "b1": b1r,
                "a1": a1,
            }
        )
    return in_maps


def kernel(features, W0, b0, W1, b1, nodes2, neigh2, neigh1, _trace=False):
    from concourse.bass_utils import run_bass_kernel_spmd

    nc = get_program()
    in_maps = prepare_in_maps(features, W0, b0, W1, b1, nodes2, neigh2, neigh1)
    kwargs = {}
    if _trace:
        import tempfile

        import ntff_shim  # noqa: F401  (registers the axon NTFF hook)

        kwargs = {"trace": True, "tmpdir": tempfile.mkdtemp(prefix="ntff_")}
    res = run_bass_kernel_spmd(nc, in_maps, list(range(N_CORES)), **kwargs)
    out = np.concatenate([res.results[c]["out"] for c in range(N_CORES)], axis=0)
    if _trace:
        _CACHE["last_result"] = res
    return out



# revision 5
# speedup vs baseline: 1.9399x; 1.9399x over previous
"""GraphSAGE-style 2-layer GNN minibatch forward on 8 trn2 NeuronCores.

Data-parallel over the 1024 target nodes: each core handles 128 targets
(1408 layer-1 tokens). Host prep resolves the neighbor indices into a
per-core bf16 feature stream laid out transposed (feature dim on
partitions, slot-major token columns); the device kernel is a pure
streaming pipeline: HWDGE DMA loads each group's [128, 3328] half-tiles,
DVE strided reduces form the neighbor sums, PE applies the MLP with the
weight chunks as stationary operands (no transposes anywhere), and the
l2-norm runs as ones-matmul (partition reduce) + rank-1 broadcast +
DVE multiply. All arithmetic (aggregation, matmuls, relu, normalize)
happens on device; the host only moves/permutes/casts bytes.

All shapes hardcoded; self-contained (only needs the concourse runtime
that ships with the container).
"""

import numpy as np

N_CORES = 8
D = 256          # feature dim
P = 128          # partitions / tokens per group
B = 1024         # total targets
S0 = 25          # layer-0 fanout
S1 = 10          # layer-1 fanout
NG = 11          # groups of 128 tokens per core at layer 1 (1408 = 11*128)
NTOK = NG * P    # 1408 layer-1 tokens per core
NCOL = P * (1 + S0)   # 3328 columns per group tile, col = s*128 + t

_CACHE = {}


def _build_program():
    import concourse.bacc as bacc
    import concourse.mybir as mybir
    import concourse.tile as tile
    from concourse.masks import make_identity

    F32 = mybir.dt.float32
    BF16 = mybir.dt.bfloat16
    AF = mybir.ActivationFunctionType
    ALU = mybir.AluOpType
    AX = mybir.AxisListType

    nc = bacc.Bacc("TRN2", target_bir_lowering=False, debug=False)

    x0t_d = nc.dram_tensor("x0t", [NG * 2, P, NCOL], BF16, kind="ExternalInput")
    w0c_d = nc.dram_tensor("w0c", [8, P, P], BF16, kind="ExternalInput")
    w1c_d = nc.dram_tensor("w1c", [8, P, P], BF16, kind="ExternalInput")
    b0h_d = nc.dram_tensor("b0h", [2, 1, P], BF16, kind="ExternalInput")
    b1h_d = nc.dram_tensor("b1h", [2, 1, P], BF16, kind="ExternalInput")
    out_d = nc.dram_tensor("out", [P, D], F32, kind="ExternalOutput")

    with tile.TileContext(nc) as tc:
        with (
            tc.tile_pool(name="consts", bufs=1) as consts,
            tc.tile_pool(name="xp", bufs=4) as xp,
            tc.tile_pool(name="agp", bufs=2) as agp,
            tc.tile_pool(name="epp", bufs=2) as epp,
            tc.tile_pool(name="psh", bufs=3, space="PSUM") as psh,
            tc.tile_pool(name="psn", bufs=2, space="PSUM") as psn,
            tc.tile_pool(name="psb", bufs=2, space="PSUM") as psb,
        ):
            ident = consts.tile([P, P], F32, tag="ident")
            make_identity(nc, ident[:])
            ones1b = consts.tile([1, P], BF16, tag="ones1b")
            nc.vector.memset(ones1b[:], 1.0)
            ones128b = consts.tile([P, 1], BF16, tag="ones128b")
            nc.vector.memset(ones128b[:], 1.0)
            ones1f = consts.tile([1, P], F32, tag="ones1f")
            nc.vector.memset(ones1f[:], 1.0)
            ones128f = consts.tile([P, 1], F32, tag="ones128f")
            nc.vector.memset(ones128f[:], 1.0)
            epsb = consts.tile([1, 1], F32, tag="epsb")
            nc.vector.memset(epsb[:], 1e-30)

            w0_sb, w1_sb = [], []
            for i in range(8):
                t0 = consts.tile([P, P], BF16, tag=f"w0c{i}")
                nc.sync.dma_start(out=t0[:], in_=w0c_d[i])
                w0_sb.append(t0)
                t1 = consts.tile([P, P], BF16, tag=f"w1c{i}")
                nc.sync.dma_start(out=t1[:], in_=w1c_d[i])
                w1_sb.append(t1)
            b0_sb, b1_sb = [], []
            for h in range(2):
                tb = consts.tile([1, P], BF16, tag=f"b0h{h}")
                nc.sync.dma_start(out=tb[:], in_=b0h_d[h])
                b0_sb.append(tb)
                tb = consts.tile([1, P], BF16, tag=f"b1h{h}")
                nc.sync.dma_start(out=tb[:], in_=b1h_d[h])
                b1_sb.append(tb)

            # h1T storage: self (group 0) and neighbor (groups 1..10) columns
            h1s = [
                consts.tile([P, P], BF16, tag=f"h1s{h}", name=f"h1s{h}")
                for h in range(2)
            ]
            h1n = [
                consts.tile([P, (NG - 1) * P], BF16, tag=f"h1n{h}", name=f"h1n{h}")
                for h in range(2)
            ]
            out_sb = consts.tile([P, D], F32, tag="out_sb")

            def mlp(x_chunks, w_sb, b_sb, dests, f32_epi):
                # dests[h] <- l2norm(relu(W.T @ x + b)) halves, transposed
                # layout [d_out, token]. x_chunks: 4 bf16 [P, P] APs (in-dim
                # chunks: self lo/hi, agg lo/hi).
                edt = F32 if f32_epi else BF16
                relus = []
                sqs = []
                for h in range(2):
                    ph = psh.tile([P, P], F32, tag="ph")
                    for k in range(4):
                        nc.tensor.matmul(
                            out=ph[:], lhsT=w_sb[k * 2 + h][:], rhs=x_chunks[k],
                            start=(k == 0), stop=False,
                        )
                    nc.tensor.matmul(
                        out=ph[:], lhsT=b_sb[h][:], rhs=ones1b[:],
                        start=False, stop=True,
                    )
                    hr = epp.tile([P, P], edt, tag=f"hr{h}")
                    nc.scalar.activation(out=hr[:], in_=ph[:], func=AF.Relu)
                    sq = epp.tile([P, P], edt, tag=f"sq{h}")
                    nc.scalar.activation(out=sq[:], in_=hr[:], func=AF.Square)
                    relus.append(hr)
                    sqs.append(sq)
                onesc = ones128f if f32_epi else ones128b
                pn = psn.tile([1, P], F32, tag="pn")
                nc.tensor.matmul(
                    out=pn[:], lhsT=onesc[:], rhs=sqs[0][:], start=True, stop=False
                )
                nc.tensor.matmul(
                    out=pn[:], lhsT=onesc[:], rhs=sqs[1][:], start=False, stop=True
                )
                nrm = epp.tile([1, P], F32, tag="nrm")
                nc.scalar.activation(
                    out=nrm[:], in_=pn[:], func=AF.Sqrt, bias=epsb[:]
                )
                rinv = epp.tile([1, P], F32, tag="rinv")
                nc.vector.reciprocal(out=rinv[:], in_=nrm[:])
                pb = psb.tile([P, P], F32, tag="pb")
                nc.tensor.matmul(
                    out=pb[:], lhsT=ones1f[:], rhs=rinv[:], start=True, stop=True
                )
                for h in range(2):
                    nc.vector.scalar_tensor_tensor(
                        out=dests[h], in0=relus[h][:], scalar=1.0, in1=pb[:],
                        op0=ALU.bypass, op1=ALU.mult,
                    )

            # ---- layer 0: 11 groups of 128 tokens, streamed ----
            for g in range(NG):
                xh = []
                for h in range(2):
                    t = xp.tile([P, NCOL], BF16, tag=f"xh{h}")
                    nc.sync.dma_start(out=t[:], in_=x0t_d[g * 2 + h])
                    xh.append(t)
                aggb = []
                for h in range(2):
                    agg = agp.tile([P, P], F32, tag=f"agg{h}")
                    nc.vector.tensor_reduce(
                        out=agg[:],
                        in_=xh[h][:, P:].rearrange("p (s t) -> p t s", s=S0),
                        axis=AX.X, op=ALU.add,
                    )
                    ab = agp.tile([P, P], BF16, tag=f"aggb{h}")
                    nc.scalar.copy(out=ab[:], in_=agg[:])
                    aggb.append(ab)
                x_chunks = [xh[0][:, 0:P], xh[1][:, 0:P], aggb[0][:], aggb[1][:]]
                if g == 0:
                    dests = [h1s[h][:] for h in range(2)]
                else:
                    dests = [h1n[h][:, (g - 1) * P:g * P] for h in range(2)]
                mlp(x_chunks, w0_sb, b0_sb, dests, f32_epi=False)

            # ---- layer 1 ----
            agg1b = []
            for h in range(2):
                agg = agp.tile([P, P], F32, tag=f"agg{h}")
                nc.vector.tensor_reduce(
                    out=agg[:],
                    in_=h1n[h][:].rearrange("p (t s) -> p t s", s=S1),
                    axis=AX.X, op=ALU.add,
                )
                ab = agp.tile([P, P], BF16, tag=f"aggb{h}")
                nc.scalar.copy(out=ab[:], in_=agg[:])
                agg1b.append(ab)
            x_chunks = [h1s[0][:], h1s[1][:], agg1b[0][:], agg1b[1][:]]
            h2 = [
                epp.tile([P, P], F32, tag=f"h2_{h}", name=f"h2_{h}")
                for h in range(2)
            ]
            mlp(x_chunks, w1_sb, b1_sb, [h2[h][:] for h in range(2)], f32_epi=True)

            # transpose h2T [d, t] -> out [t, d] and write
            for h in range(2):
                tp = psb.tile([P, P], F32, tag="pb")
                nc.tensor.transpose(out=tp[:], in_=h2[h][:], identity=ident[:])
                nc.scalar.copy(out=out_sb[:, h * P:(h + 1) * P], in_=tp[:])
            nc.sync.dma_start(out=out_d[:], in_=out_sb[:])

    nc.compile()
    return nc


def get_program():
    if "nc" not in _CACHE:
        _CACHE["nc"] = _build_program()
    return _CACHE["nc"]


def prepare_in_maps(features, W0, b0, W1, b1, nodes2, neigh2, neigh1):
    """Host-side sharding + index-resolved bf16 stream layout + weight prep."""
    import ml_dtypes

    bf16 = ml_dtypes.bfloat16
    feats16 = np.asarray(features, dtype=np.float32).astype(bf16)

    def chunk_w(W, fan):
        wt = np.ascontiguousarray(np.asarray(W, dtype=np.float32).T).copy()
        wt[D:, :] /= fan  # fold the neighbor mean into the weights
        # [k*2+h] = wt[k*128:(k+1)*128, h*128:(h+1)*128]
        return np.ascontiguousarray(
            wt.reshape(4, P, 2, P).transpose(0, 2, 1, 3).reshape(8, P, P)
        ).astype(bf16)

    w0c = chunk_w(W0, S0)
    w1c = chunk_w(W1, S1)
    b0h = np.asarray(b0, dtype=np.float32).reshape(2, 1, P).astype(bf16)
    b1h = np.asarray(b1, dtype=np.float32).reshape(2, 1, P).astype(bf16)

    in_maps = []
    bc = B // N_CORES  # 128 targets per core
    for c in range(N_CORES):
        nodes2_c = nodes2[c * bc:(c + 1) * bc]
        neigh2_c = neigh2[c * bc:(c + 1) * bc, :]
        nodes1_c = np.concatenate([nodes2_c, neigh2_c.reshape(-1)])
        neigh1_c = np.concatenate(
            [
                neigh1[c * bc:(c + 1) * bc, :],
                neigh1[B + c * bc * S1:B + (c + 1) * bc * S1, :],
            ],
            axis=0,
        )
        idx0_c = np.concatenate([nodes1_c[:, None], neigh1_c], axis=1)  # [1408, 26]
        # per group slot-major columns: col = s*128 + t
        flat = idx0_c.reshape(NG, P, 1 + S0).transpose(0, 2, 1).reshape(-1)
        rows = feats16[flat].reshape(NG, NCOL, D)
        x0t = np.ascontiguousarray(
            rows.transpose(0, 2, 1).reshape(NG, 2, P, NCOL)
        ).reshape(NG * 2, P, NCOL)
        in_maps.append(
            {"x0t": x0t, "w0c": w0c, "w1c": w1c, "b0h": b0h, "b1h": b1h}
        )
    return in_maps


def kernel(features, W0, b0, W1, b1, nodes2, neigh2, neigh1, _trace=False):
    from concourse.bass_utils import run_bass_kernel_spmd

    nc = get_program()
    in_maps = prepare_in_maps(features, W0, b0, W1, b1, nodes2, neigh2, neigh1)
    kwargs = {}
    if _trace:
        import tempfile

        import ntff_shim  # noqa: F401  (registers the axon NTFF hook)

        kwargs = {"trace": True, "tmpdir": tempfile.mkdtemp(prefix="ntff_")}
    res = run_bass_kernel_spmd(nc, in_maps, list(range(N_CORES)), **kwargs)
    out = np.concatenate([res.results[c]["out"] for c in range(N_CORES)], axis=0)
    if _trace:
        _CACHE["last_result"] = res
    return out


# revision 6
# speedup vs baseline: 2.5114x; 1.2946x over previous
"""GraphSAGE-style 2-layer GNN minibatch forward on 8 trn2 NeuronCores.

Data-parallel over the 1024 target nodes: each core handles 128 targets
(1408 layer-1 tokens). Host prep resolves the neighbor indices into a
per-core bf16 feature stream laid out transposed (feature dim on
partitions; per group: 128 self columns, then 128x25 token-major
neighbor columns so the DVE reduce reads contiguous runs); the device
kernel is a pure streaming pipeline: HWDGE DMA loads each group's
[128, 3328] half-tiles, DVE strided reduces form the neighbor sums, PE
applies the MLP with the weight chunks as stationary operands (no
transposes anywhere), and the l2-norm is batched: per-group ones-matmul
norms, then one sqrt + reciprocal + rank-1 broadcast + DVE multiply
pass over all 1408 tokens. All arithmetic (aggregation, matmuls, relu,
normalize) happens on device; the host only moves/permutes/casts bytes.

All shapes hardcoded; self-contained (only needs the concourse runtime
that ships with the container).
"""

import numpy as np

N_CORES = 8
D = 256          # feature dim
P = 128          # partitions / tokens per group
B = 1024         # total targets
S0 = 25          # layer-0 fanout
S1 = 10          # layer-1 fanout
NG = 11          # groups of 128 tokens per core at layer 1 (1408 = 11*128)
NTOK = NG * P    # 1408 layer-1 tokens per core
NCOL = P * (1 + S0)   # 3328 columns per group tile: [self 128 | t-major 128*25]

_CACHE = {}


def _build_program():
    import concourse.bacc as bacc
    import concourse.mybir as mybir
    import concourse.tile as tile
    from concourse.masks import make_identity

    F32 = mybir.dt.float32
    BF16 = mybir.dt.bfloat16
    AF = mybir.ActivationFunctionType
    ALU = mybir.AluOpType
    AX = mybir.AxisListType

    nc = bacc.Bacc("TRN2", target_bir_lowering=False, debug=False)

    x0t_d = nc.dram_tensor("x0t", [NG * 2, P, NCOL], BF16, kind="ExternalInput")
    w0c_d = nc.dram_tensor("w0c", [8, P, P], BF16, kind="ExternalInput")
    w1c_d = nc.dram_tensor("w1c", [8, P, P], BF16, kind="ExternalInput")
    b0h_d = nc.dram_tensor("b0h", [2, 1, P], BF16, kind="ExternalInput")
    b1h_d = nc.dram_tensor("b1h", [2, 1, P], BF16, kind="ExternalInput")
    out_d = nc.dram_tensor("out", [P, D], F32, kind="ExternalOutput")

    # batched-normalize column chunks covering the 1408 token columns
    NCH = [(0, 512), (512, 512), (1024, 384)]

    with tile.TileContext(nc) as tc:
        with (
            tc.tile_pool(name="consts", bufs=1) as consts,
            tc.tile_pool(name="xp", bufs=4) as xp,
            tc.tile_pool(name="agp", bufs=2) as agp,
            tc.tile_pool(name="epp", bufs=2) as epp,
            tc.tile_pool(name="psh", bufs=3, space="PSUM") as psh,
            tc.tile_pool(name="psn", bufs=2, space="PSUM") as psn,
            tc.tile_pool(name="psb", bufs=2, space="PSUM") as psb,
        ):
            ident = consts.tile([P, P], F32, tag="ident")
            make_identity(nc, ident[:])
            ones1b = consts.tile([1, P], BF16, tag="ones1b")
            nc.vector.memset(ones1b[:], 1.0)
            ones128b = consts.tile([P, 1], BF16, tag="ones128b")
            nc.vector.memset(ones128b[:], 1.0)
            ones1f = consts.tile([1, P], F32, tag="ones1f")
            nc.vector.memset(ones1f[:], 1.0)
            ones128f = consts.tile([P, 1], F32, tag="ones128f")
            nc.vector.memset(ones128f[:], 1.0)
            epsb = consts.tile([1, 1], F32, tag="epsb")
            nc.vector.memset(epsb[:], 1e-30)

            w0_sb, w1_sb = [], []
            for i in range(8):
                t0 = consts.tile([P, P], BF16, tag=f"w0c{i}")
                nc.sync.dma_start(out=t0[:], in_=w0c_d[i])
                w0_sb.append(t0)
                t1 = consts.tile([P, P], BF16, tag=f"w1c{i}")
                nc.sync.dma_start(out=t1[:], in_=w1c_d[i])
                w1_sb.append(t1)
            b0_sb, b1_sb = [], []
            for h in range(2):
                tb = consts.tile([1, P], BF16, tag=f"b0h{h}")
                nc.sync.dma_start(out=tb[:], in_=b0h_d[h])
                b0_sb.append(tb)
                tb = consts.tile([1, P], BF16, tag=f"b1h{h}")
                nc.sync.dma_start(out=tb[:], in_=b1h_d[h])
                b1_sb.append(tb)

            # layer-0 outputs: unnormalized relu h (transposed), and norms
            hr_all = [
                consts.tile([P, NTOK], BF16, tag=f"hr_all{h}", name=f"hr_all{h}")
                for h in range(2)
            ]
            h1_all = [
                consts.tile([P, NTOK], BF16, tag=f"h1_all{h}", name=f"h1_all{h}")
                for h in range(2)
            ]
            norms = consts.tile([1, NTOK], F32, tag="norms")
            out_sb = consts.tile([P, D], F32, tag="out_sb")

            # ---- layer 0: 11 groups of 128 tokens, streamed ----
            for g in range(NG):
                xh = []
                for h in range(2):
                    t = xp.tile([P, NCOL], BF16, tag=f"xh{h}")
                    nc.sync.dma_start(out=t[:], in_=x0t_d[g * 2 + h])
                    xh.append(t)
                aggb = []
                for h in range(2):
                    agg = agp.tile([P, P], F32, tag=f"agg{h}")
                    nc.vector.tensor_reduce(
                        out=agg[:],
                        in_=xh[h][:, P:].rearrange("p (t s) -> p t s", s=S0),
                        axis=AX.X, op=ALU.add,
                    )
                    ab = agp.tile([P, P], BF16, tag=f"aggb{h}")
                    nc.scalar.copy(out=ab[:], in_=agg[:])
                    aggb.append(ab)
                x_chunks = [xh[0][:, 0:P], xh[1][:, 0:P], aggb[0][:], aggb[1][:]]
                sqs = []
                for h in range(2):
                    ph = psh.tile([P, P], F32, tag="ph")
                    for k in range(4):
                        nc.tensor.matmul(
                            out=ph[:], lhsT=w0_sb[k * 2 + h][:], rhs=x_chunks[k],
                            start=(k == 0), stop=False,
                        )
                    nc.tensor.matmul(
                        out=ph[:], lhsT=b0_sb[h][:], rhs=ones1b[:],
                        start=False, stop=True,
                    )
                    hr = hr_all[h][:, g * P:(g + 1) * P]
                    nc.scalar.activation(out=hr, in_=ph[:], func=AF.Relu)
                    sq = epp.tile([P, P], BF16, tag=f"sq{h}")
                    nc.scalar.activation(out=sq[:], in_=hr, func=AF.Square)
                    sqs.append(sq)
                pn = psn.tile([1, P], F32, tag="pn")
                nc.tensor.matmul(
                    out=pn[:], lhsT=ones128b[:], rhs=sqs[0][:], start=True, stop=False
                )
                nc.tensor.matmul(
                    out=pn[:], lhsT=ones128b[:], rhs=sqs[1][:], start=False, stop=True
                )
                nc.scalar.copy(out=norms[:, g * P:(g + 1) * P], in_=pn[:])

            # ---- batched l2-normalize of all 1408 layer-1 tokens ----
            nrm = epp.tile([1, NTOK], F32, tag="nrm", name="nrm_l0")
            nc.scalar.activation(out=nrm[:], in_=norms[:], func=AF.Sqrt, bias=epsb[:])
            rinv = epp.tile([1, NTOK], F32, tag="rinv", name="rinv_l0")
            nc.vector.reciprocal(out=rinv[:], in_=nrm[:])
            for c0, cn in NCH:
                pb = psb.tile([P, 512], F32, tag="pb")
                nc.tensor.matmul(
                    out=pb[:, 0:cn], lhsT=ones1f[:], rhs=rinv[:, c0:c0 + cn],
                    start=True, stop=True,
                )
                for h in range(2):
                    nc.vector.scalar_tensor_tensor(
                        out=h1_all[h][:, c0:c0 + cn],
                        in0=hr_all[h][:, c0:c0 + cn], scalar=1.0, in1=pb[:, 0:cn],
                        op0=ALU.bypass, op1=ALU.mult,
                    )

            # ---- layer 1 ----
            agg1b = []
            for h in range(2):
                agg = agp.tile([P, P], F32, tag=f"agg{h}")
                nc.vector.tensor_reduce(
                    out=agg[:],
                    in_=h1_all[h][:, P:].rearrange("p (t s) -> p t s", s=S1),
                    axis=AX.X, op=ALU.add,
                )
                ab = agp.tile([P, P], BF16, tag=f"aggb{h}")
                nc.scalar.copy(out=ab[:], in_=agg[:])
                agg1b.append(ab)
            x_chunks = [
                h1_all[0][:, 0:P], h1_all[1][:, 0:P], agg1b[0][:], agg1b[1][:]
            ]
            relus2, sqs2 = [], []
            for h in range(2):
                ph = psh.tile([P, P], F32, tag="ph")
                for k in range(4):
                    nc.tensor.matmul(
                        out=ph[:], lhsT=w1_sb[k * 2 + h][:], rhs=x_chunks[k],
                        start=(k == 0), stop=False,
                    )
                nc.tensor.matmul(
                    out=ph[:], lhsT=b1_sb[h][:], rhs=ones1b[:], start=False, stop=True
                )
                hr = epp.tile([P, P], F32, tag=f"hr2_{h}", name=f"hr2_{h}")
                nc.scalar.activation(out=hr[:], in_=ph[:], func=AF.Relu)
                sq = epp.tile([P, P], F32, tag=f"sq2_{h}", name=f"sq2_{h}")
                nc.scalar.activation(out=sq[:], in_=hr[:], func=AF.Square)
                relus2.append(hr)
                sqs2.append(sq)
            pn = psn.tile([1, P], F32, tag="pn")
            nc.tensor.matmul(
                out=pn[:], lhsT=ones128f[:], rhs=sqs2[0][:], start=True, stop=False
            )
            nc.tensor.matmul(
                out=pn[:], lhsT=ones128f[:], rhs=sqs2[1][:], start=False, stop=True
            )
            nrm2 = epp.tile([1, P], F32, tag="nrm2", name="nrm_l1")
            nc.scalar.activation(out=nrm2[:], in_=pn[:], func=AF.Sqrt, bias=epsb[:])
            rinv2 = epp.tile([1, P], F32, tag="rinv2", name="rinv_l1")
            nc.vector.reciprocal(out=rinv2[:], in_=nrm2[:])
            pb2 = psb.tile([P, 512], F32, tag="pb")
            nc.tensor.matmul(
                out=pb2[:, 0:P], lhsT=ones1f[:], rhs=rinv2[:], start=True, stop=True
            )
            # scale + transpose h2T [d, t] -> out [t, d] and write
            for h in range(2):
                h2 = epp.tile([P, P], F32, tag=f"h2_{h}", name=f"h2_{h}")
                nc.vector.scalar_tensor_tensor(
                    out=h2[:], in0=relus2[h][:], scalar=1.0, in1=pb2[:, 0:P],
                    op0=ALU.bypass, op1=ALU.mult,
                )
                tp = psb.tile([P, 512], F32, tag="pb")
                nc.tensor.transpose(out=tp[:, 0:P], in_=h2[:], identity=ident[:])
                nc.scalar.copy(out=out_sb[:, h * P:(h + 1) * P], in_=tp[:, 0:P])
            nc.sync.dma_start(out=out_d[:], in_=out_sb[:])

    nc.compile()
    return nc


def get_program():
    if "nc" not in _CACHE:
        _CACHE["nc"] = _build_program()
    return _CACHE["nc"]


def prepare_in_maps(features, W0, b0, W1, b1, nodes2, neigh2, neigh1):
    """Host-side sharding + index-resolved bf16 stream layout + weight prep."""
    import ml_dtypes

    bf16 = ml_dtypes.bfloat16
    feats16 = np.asarray(features, dtype=np.float32).astype(bf16)

    def chunk_w(W, fan):
        wt = np.ascontiguousarray(np.asarray(W, dtype=np.float32).T).copy()
        wt[D:, :] /= fan  # fold the neighbor mean into the weights
        # [k*2+h] = wt[k*128:(k+1)*128, h*128:(h+1)*128]
        return np.ascontiguousarray(
            wt.reshape(4, P, 2, P).transpose(0, 2, 1, 3).reshape(8, P, P)
        ).astype(bf16)

    w0c = chunk_w(W0, S0)
    w1c = chunk_w(W1, S1)
    b0h = np.asarray(b0, dtype=np.float32).reshape(2, 1, P).astype(bf16)
    b1h = np.asarray(b1, dtype=np.float32).reshape(2, 1, P).astype(bf16)

    in_maps = []
    bc = B // N_CORES  # 128 targets per core
    for c in range(N_CORES):
        nodes2_c = nodes2[c * bc:(c + 1) * bc]
        neigh2_c = neigh2[c * bc:(c + 1) * bc, :]
        nodes1_c = np.concatenate([nodes2_c, neigh2_c.reshape(-1)])
        neigh1_c = np.concatenate(
            [
                neigh1[c * bc:(c + 1) * bc, :],
                neigh1[B + c * bc * S1:B + (c + 1) * bc * S1, :],
            ],
            axis=0,
        )
        # per group columns: [128 self | 128*25 token-major neighbors]
        nodes1_g = nodes1_c.reshape(NG, P)
        neigh1_g = neigh1_c.reshape(NG, P * S0)
        flat = np.concatenate([nodes1_g, neigh1_g], axis=1).reshape(-1)
        rows = feats16[flat].reshape(NG, NCOL, D)
        x0t = np.ascontiguousarray(
            rows.transpose(0, 2, 1).reshape(NG, 2, P, NCOL)
        ).reshape(NG * 2, P, NCOL)
        in_maps.append(
            {"x0t": x0t, "w0c": w0c, "w1c": w1c, "b0h": b0h, "b1h": b1h}
        )
    return in_maps


def kernel(features, W0, b0, W1, b1, nodes2, neigh2, neigh1, _trace=False):
    from concourse.bass_utils import run_bass_kernel_spmd

    nc = get_program()
    in_maps = prepare_in_maps(features, W0, b0, W1, b1, nodes2, neigh2, neigh1)
    kwargs = {}
    if _trace:
        import tempfile

        import ntff_shim  # noqa: F401  (registers the axon NTFF hook)

        kwargs = {"trace": True, "tmpdir": tempfile.mkdtemp(prefix="ntff_")}
    res = run_bass_kernel_spmd(nc, in_maps, list(range(N_CORES)), **kwargs)
    out = np.concatenate([res.results[c]["out"] for c in range(N_CORES)], axis=0)
    if _trace:
        _CACHE["last_result"] = res
    return out


# revision 10
# speedup vs baseline: 2.7394x; 1.0908x over previous
"""GraphSAGE-style 2-layer GNN minibatch forward on 8 trn2 NeuronCores.

Data-parallel over the 1024 target nodes: each core handles 128 targets
(1408 layer-1 tokens). Host prep resolves the neighbor indices into a
per-core bf16 feature stream laid out transposed (feature dim on
partitions; per group: 128 self columns, then 25 slot-major [128-token]
neighbor slabs); the device kernel is a pure streaming pipeline: HWDGE
DMA loads each group's [128, 3328] half-tiles, the 25-neighbor sum runs
as a 6-op bf16 tensor-tensor ADD tree on DVE (4x perf mode eligible,
unlike tensor_reduce), PE applies the MLP with the weight chunks as
stationary operands (no data transposes), and the l2-norm is batched:
per-group Gram-column matmuls produce [128,1] sq-norms, one sqrt +
reciprocal over [128, NG], a single PE transpose, then per-group rank-1
broadcast + DVE multiply. All arithmetic (aggregation, matmuls, relu,
normalize) happens on device; the host only moves/permutes/casts bytes.

All shapes hardcoded; self-contained (only needs the concourse runtime
that ships with the container).
"""

import numpy as np

N_CORES = 8
D = 256          # feature dim
P = 128          # partitions / tokens per group
B = 1024         # total targets
S0 = 25          # layer-0 fanout
S1 = 10          # layer-1 fanout
NG = 11          # groups of 128 tokens per core at layer 1 (1408 = 11*128)
NTOK = NG * P    # 1408 layer-1 tokens per core
NCOL = P * (1 + S0)   # 3328 columns per group tile: [self 128 | 25 slabs of 128]

_CACHE = {}


def _build_program():
    import concourse.bacc as bacc
    import concourse.mybir as mybir
    import concourse.tile as tile
    from concourse.masks import make_identity

    F32 = mybir.dt.float32
    BF16 = mybir.dt.bfloat16
    AF = mybir.ActivationFunctionType
    ALU = mybir.AluOpType
    AX = mybir.AxisListType

    nc = bacc.Bacc("TRN2", target_bir_lowering=False, debug=False)

    x0t_d = nc.dram_tensor("x0t", [NG * 2, P, NCOL], BF16, kind="ExternalInput")
    w0c_d = nc.dram_tensor("w0c", [8, P, P], BF16, kind="ExternalInput")
    w1c_d = nc.dram_tensor("w1c", [8, P, P], BF16, kind="ExternalInput")
    b0h_d = nc.dram_tensor("b0h", [2, 1, P], BF16, kind="ExternalInput")
    b1h_d = nc.dram_tensor("b1h", [2, 1, P], BF16, kind="ExternalInput")
    out_d = nc.dram_tensor("out", [P, D], F32, kind="ExternalOutput")

    with tile.TileContext(nc) as tc:
        with (
            tc.tile_pool(name="consts", bufs=1) as consts,
            tc.tile_pool(name="xp", bufs=4) as xp,
            tc.tile_pool(name="trp", bufs=2) as trp,
            tc.tile_pool(name="epp", bufs=2) as epp,
            tc.tile_pool(name="psh", bufs=3, space="PSUM") as psh,
            tc.tile_pool(name="psn", bufs=1, space="PSUM") as psn,
            tc.tile_pool(name="psb", bufs=2, space="PSUM") as psb,
        ):
            ident = consts.tile([P, P], F32, tag="ident")
            make_identity(nc, ident[:])
            ones1b = consts.tile([1, P], BF16, tag="ones1b")
            nc.vector.memset(ones1b[:], 1.0)
            ones128b = consts.tile([P, 1], BF16, tag="ones128b")
            nc.vector.memset(ones128b[:], 1.0)
            ones1f = consts.tile([1, P], F32, tag="ones1f")
            nc.vector.memset(ones1f[:], 1.0)
            ones128f = consts.tile([P, 1], F32, tag="ones128f")
            nc.vector.memset(ones128f[:], 1.0)
            epsb = consts.tile([1, 1], F32, tag="epsb")
            nc.vector.memset(epsb[:], 1e-30)
            epsp = consts.tile([P, 1], F32, tag="epsp")
            nc.vector.memset(epsp[:], 1e-30)

            w0_sb, w1_sb = [], []
            for i in range(8):
                t0 = consts.tile([P, P], BF16, tag=f"w0c{i}")
                nc.sync.dma_start(out=t0[:], in_=w0c_d[i])
                w0_sb.append(t0)
                t1 = consts.tile([P, P], BF16, tag=f"w1c{i}")
                nc.sync.dma_start(out=t1[:], in_=w1c_d[i])
                w1_sb.append(t1)
            b0_sb, b1_sb = [], []
            for h in range(2):
                tb = consts.tile([1, P], BF16, tag=f"b0h{h}")
                nc.sync.dma_start(out=tb[:], in_=b0h_d[h])
                b0_sb.append(tb)
                tb = consts.tile([1, P], BF16, tag=f"b1h{h}")
                nc.sync.dma_start(out=tb[:], in_=b1h_d[h])
                b1_sb.append(tb)

            # layer-0 outputs: unnormalized relu h (transposed), and sq-norms
            hr_all = [
                consts.tile([P, NTOK], BF16, tag=f"hr_all{h}", name=f"hr_all{h}")
                for h in range(2)
            ]
            h1_all = [
                consts.tile([P, NTOK], BF16, tag=f"h1_all{h}", name=f"h1_all{h}")
                for h in range(2)
            ]
            norms_t = consts.tile([P, NG], F32, tag="norms_t")
            out_sb = consts.tile([P, D], F32, tag="out_sb")

            def agg_tree(xn, tag):
                # xn: [P, 25*P] bf16, 25 slot-major slabs; returns bf16 [P, P]
                # sum via tensor-tensor adds (fast-mode eligible on DVE).
                t1 = trp.tile([P, 12 * P], BF16, tag=f"t1{tag}")
                nc.vector.tensor_add(t1[:], xn[:, 0:12 * P], xn[:, 12 * P:24 * P])
                t2 = trp.tile([P, 6 * P], BF16, tag=f"t2{tag}")
                nc.vector.tensor_add(t2[:], t1[:, 0:6 * P], t1[:, 6 * P:12 * P])
                t3 = trp.tile([P, 3 * P], BF16, tag=f"t3{tag}")
                nc.vector.tensor_add(t3[:], t2[:, 0:3 * P], t2[:, 3 * P:6 * P])
                t4 = trp.tile([P, P], BF16, tag=f"t4{tag}")
                nc.vector.tensor_add(t4[:], t3[:, 0:P], t3[:, P:2 * P])
                t5 = trp.tile([P, P], BF16, tag=f"t5{tag}")
                nc.vector.tensor_add(t5[:], t4[:], t3[:, 2 * P:3 * P])
                ab = trp.tile([P, P], BF16, tag=f"ab{tag}")
                nc.vector.tensor_add(ab[:], t5[:], xn[:, 24 * P:25 * P])
                return ab

            # ---- layer 0: 11 groups of 128 tokens, streamed ----
            for g in range(NG):
                xh = []
                for h in range(2):
                    t = xp.tile([P, NCOL], BF16, tag=f"xh{h}")
                    nc.sync.dma_start(out=t[:], in_=x0t_d[g * 2 + h])
                    xh.append(t)
                aggb = [agg_tree(xh[h][:, P:], h) for h in range(2)]
                x_chunks = [xh[0][:, 0:P], xh[1][:, 0:P], aggb[0][:], aggb[1][:]]
                sqs = []
                for h in range(2):
                    ph = psh.tile([P, P], F32, tag="ph")
                    for k in range(4):
                        nc.tensor.matmul(
                            out=ph[:], lhsT=w0_sb[k * 2 + h][:], rhs=x_chunks[k],
                            start=(k == 0), stop=False,
                        )
                    nc.tensor.matmul(
                        out=ph[:], lhsT=b0_sb[h][:], rhs=ones1b[:],
                        start=False, stop=True,
                    )
                    hr = hr_all[h][:, g * P:(g + 1) * P]
                    nc.scalar.activation(out=hr, in_=ph[:], func=AF.Relu)
                    sq = epp.tile([P, P], BF16, tag=f"sq{h}")
                    nc.scalar.activation(out=sq[:], in_=hr, func=AF.Square)
                    sqs.append(sq)
                pn = psn.tile([P, 1], F32, tag="pn")
                nc.tensor.matmul(
                    out=pn[:], lhsT=sqs[0][:], rhs=ones128b[:], start=True, stop=False
                )
                nc.tensor.matmul(
                    out=pn[:], lhsT=sqs[1][:], rhs=ones128b[:], start=False, stop=True
                )
                nc.scalar.copy(out=norms_t[:, g:g + 1], in_=pn[:])

            # ---- batched l2-normalize of all 1408 layer-1 tokens ----
            nrm = epp.tile([P, NG], F32, tag="nrm", name="nrm_l0")
            nc.scalar.activation(
                out=nrm[:], in_=norms_t[:], func=AF.Sqrt, bias=epsp[:]
            )
            rinv_t = epp.tile([P, NG], F32, tag="rinv", name="rinv_l0")
            nc.vector.reciprocal(out=rinv_t[:], in_=nrm[:])
            for g in range(NG):
                # transpose rinv column g to a [1, P] row, then rank-1 expand
                pr = psn.tile([1, P], F32, tag="pr")
                nc.tensor.matmul(
                    out=pr[:], lhsT=rinv_t[:, g:g + 1], rhs=ident[:],
                    start=True, stop=True,
                )
                rrow = epp.tile([1, P], F32, tag="rrow")
                nc.scalar.copy(out=rrow[:], in_=pr[:])
                pb = psb.tile([P, P], F32, tag="pb")
                nc.tensor.matmul(
                    out=pb[:], lhsT=ones1f[:], rhs=rrow[:],
                    start=True, stop=True,
                )
                for h in range(2):
                    nc.vector.scalar_tensor_tensor(
                        out=h1_all[h][:, g * P:(g + 1) * P],
                        in0=hr_all[h][:, g * P:(g + 1) * P], scalar=1.0, in1=pb[:],
                        op0=ALU.bypass, op1=ALU.mult,
                    )

            # ---- layer 1 ----
            agg1b = []
            for h in range(2):
                agg = trp.tile([P, P], F32, tag=f"agg1{h}")
                nc.vector.tensor_reduce(
                    out=agg[:],
                    in_=h1_all[h][:, P:].rearrange("p (t s) -> p t s", s=S1),
                    axis=AX.X, op=ALU.add,
                )
                ab = trp.tile([P, P], BF16, tag=f"agg1b{h}")
                nc.scalar.copy(out=ab[:], in_=agg[:])
                agg1b.append(ab)
            x_chunks = [
                h1_all[0][:, 0:P], h1_all[1][:, 0:P], agg1b[0][:], agg1b[1][:]
            ]
            relus2, sqs2 = [], []
            for h in range(2):
                ph = psh.tile([P, P], F32, tag="ph")
                for k in range(4):
                    nc.tensor.matmul(
                        out=ph[:], lhsT=w1_sb[k * 2 + h][:], rhs=x_chunks[k],
                        start=(k == 0), stop=False,
                    )
                nc.tensor.matmul(
                    out=ph[:], lhsT=b1_sb[h][:], rhs=ones1b[:], start=False, stop=True
                )
                hr = epp.tile([P, P], F32, tag=f"hr2_{h}", name=f"hr2_{h}")
                nc.scalar.activation(out=hr[:], in_=ph[:], func=AF.Relu)
                sq = epp.tile([P, P], F32, tag=f"sq2_{h}", name=f"sq2_{h}")
                nc.scalar.activation(out=sq[:], in_=hr[:], func=AF.Square)
                relus2.append(hr)
                sqs2.append(sq)
            pn2 = psn.tile([1, P], F32, tag="pn2")
            nc.tensor.matmul(
                out=pn2[:], lhsT=ones128f[:], rhs=sqs2[0][:], start=True, stop=False
            )
            nc.tensor.matmul(
                out=pn2[:], lhsT=ones128f[:], rhs=sqs2[1][:], start=False, stop=True
            )
            nrm2 = epp.tile([1, P], F32, tag="nrm2", name="nrm_l1")
            nc.scalar.activation(out=nrm2[:], in_=pn2[:], func=AF.Sqrt, bias=epsb[:])
            rinv2 = epp.tile([1, P], F32, tag="rinv2", name="rinv_l1")
            nc.vector.reciprocal(out=rinv2[:], in_=nrm2[:])
            pb2 = psb.tile([P, P], F32, tag="pb")
            nc.tensor.matmul(
                out=pb2[:], lhsT=ones1f[:], rhs=rinv2[:], start=True, stop=True
            )
            # scale + transpose h2T [d, t] -> out [t, d] and write
            for h in range(2):
                h2 = epp.tile([P, P], F32, tag=f"h2_{h}", name=f"h2_{h}")
                nc.vector.scalar_tensor_tensor(
                    out=h2[:], in0=relus2[h][:], scalar=1.0, in1=pb2[:],
                    op0=ALU.bypass, op1=ALU.mult,
                )
                tp = psb.tile([P, P], F32, tag="pb")
                nc.tensor.transpose(out=tp[:], in_=h2[:], identity=ident[:])
                nc.scalar.copy(out=out_sb[:, h * P:(h + 1) * P], in_=tp[:])
            nc.sync.dma_start(out=out_d[:], in_=out_sb[:])

    nc.compile()
    return nc


def get_program():
    if "nc" not in _CACHE:
        _CACHE["nc"] = _build_program()
    return _CACHE["nc"]


def prepare_in_maps(features, W0, b0, W1, b1, nodes2, neigh2, neigh1):
    """Host-side sharding + index-resolved bf16 stream layout + weight prep."""
    import ml_dtypes

    bf16 = ml_dtypes.bfloat16
    feats16 = np.asarray(features, dtype=np.float32).astype(bf16)

    def chunk_w(W, fan):
        wt = np.ascontiguousarray(np.asarray(W, dtype=np.float32).T).copy()
        wt[D:, :] /= fan  # fold the neighbor mean into the weights
        # [k*2+h] = wt[k*128:(k+1)*128, h*128:(h+1)*128]
        return np.ascontiguousarray(
            wt.reshape(4, P, 2, P).transpose(0, 2, 1, 3).reshape(8, P, P)
        ).astype(bf16)

    w0c = chunk_w(W0, S0)
    w1c = chunk_w(W1, S1)
    b0h = np.asarray(b0, dtype=np.float32).reshape(2, 1, P).astype(bf16)
    b1h = np.asarray(b1, dtype=np.float32).reshape(2, 1, P).astype(bf16)

    in_maps = []
    bc = B // N_CORES  # 128 targets per core
    for c in range(N_CORES):
        nodes2_c = nodes2[c * bc:(c + 1) * bc]
        neigh2_c = neigh2[c * bc:(c + 1) * bc, :]
        nodes1_c = np.concatenate([nodes2_c, neigh2_c.reshape(-1)])
        neigh1_c = np.concatenate(
            [
                neigh1[c * bc:(c + 1) * bc, :],
                neigh1[B + c * bc * S1:B + (c + 1) * bc * S1, :],
            ],
            axis=0,
        )
        # per group columns: [128 self | 25 slot-major slabs of 128 tokens]
        nodes1_g = nodes1_c.reshape(NG, P)
        neigh1_g = neigh1_c.reshape(NG, P, S0).transpose(0, 2, 1).reshape(NG, P * S0)
        flat = np.concatenate([nodes1_g, neigh1_g], axis=1).reshape(-1)
        rows = feats16[flat].reshape(NG, NCOL, D)
        x0t = np.ascontiguousarray(
            rows.transpose(0, 2, 1).reshape(NG, 2, P, NCOL)
        ).reshape(NG * 2, P, NCOL)
        in_maps.append(
            {"x0t": x0t, "w0c": w0c, "w1c": w1c, "b0h": b0h, "b1h": b1h}
        )
    return in_maps


def kernel(features, W0, b0, W1, b1, nodes2, neigh2, neigh1, _trace=False):
    from concourse.bass_utils import run_bass_kernel_spmd

    nc = get_program()
    in_maps = prepare_in_maps(features, W0, b0, W1, b1, nodes2, neigh2, neigh1)
    kwargs = {}
    if _trace:
        import tempfile

        import ntff_shim  # noqa: F401  (registers the axon NTFF hook)

        kwargs = {"trace": True, "tmpdir": tempfile.mkdtemp(prefix="ntff_")}
    res = run_bass_kernel_spmd(nc, in_maps, list(range(N_CORES)), **kwargs)
    out = np.concatenate([res.results[c]["out"] for c in range(N_CORES)], axis=0)
    if _trace:
        _CACHE["last_result"] = res
    return out


# revision 13
# speedup vs baseline: 2.8016x; 1.0227x over previous
"""GraphSAGE-style 2-layer GNN minibatch forward on 8 trn2 NeuronCores.

Data-parallel over the 1024 target nodes: each core handles 128 targets
(1408 layer-1 tokens). Host prep resolves the neighbor indices into a
per-core bf16 feature stream laid out transposed (feature dim on
partitions; per group: 128 self columns, then 25 slot-major [128-token]
neighbor slabs); the device kernel is a pure streaming pipeline: HWDGE
DMA loads each group's [128, 3328] half-tiles, the 25-neighbor sum runs
as a 6-op bf16 tensor-tensor ADD tree on DVE (fast-mode eligible,
unlike tensor_reduce), PE applies the MLP with the weight chunks as
stationary operands (no data transposes), and the l2-norm runs
per-group in the wide [128,1] orientation (Gram-column matmul, sqrt,
reciprocal, PE transpose to a row, rank-1 broadcast, DVE multiply) so
it overlaps the DMA-paced loop. All arithmetic (aggregation, matmuls,
relu, normalize) happens on device; the host only moves/permutes/casts
bytes.

All shapes hardcoded; self-contained (only needs the concourse runtime
that ships with the container).
"""

import numpy as np

N_CORES = 8
D = 256          # feature dim
P = 128          # partitions / tokens per group
B = 1024         # total targets
S0 = 25          # layer-0 fanout
S1 = 10          # layer-1 fanout
NG = 11          # groups of 128 tokens per core at layer 1 (1408 = 11*128)
NTOK = NG * P    # 1408 layer-1 tokens per core
NCOL = P * (1 + S0)   # 3328 columns per group tile: [self 128 | 25 slabs of 128]

_CACHE = {}


def _build_program():
    import concourse.bacc as bacc
    import concourse.mybir as mybir
    import concourse.tile as tile
    from concourse.masks import make_identity

    F32 = mybir.dt.float32
    BF16 = mybir.dt.bfloat16
    AF = mybir.ActivationFunctionType
    ALU = mybir.AluOpType
    AX = mybir.AxisListType

    nc = bacc.Bacc("TRN2", target_bir_lowering=False, debug=False)

    x0t_d = nc.dram_tensor("x0t", [NG * 2, P, NCOL], BF16, kind="ExternalInput")
    wc_d = nc.dram_tensor("wc", [P, 16 * P], BF16, kind="ExternalInput")
    bh_d = nc.dram_tensor("bh", [1, 4 * P], BF16, kind="ExternalInput")
    out_d = nc.dram_tensor("out", [P, D], F32, kind="ExternalOutput")

    with tile.TileContext(nc) as tc:
        with (
            tc.tile_pool(name="consts", bufs=1) as consts,
            tc.tile_pool(name="xp", bufs=4) as xp,
            tc.tile_pool(name="trp", bufs=2) as trp,
            tc.tile_pool(name="epp", bufs=2) as epp,
            tc.tile_pool(name="psh", bufs=3, space="PSUM") as psh,
            tc.tile_pool(name="psn", bufs=2, space="PSUM") as psn,
            tc.tile_pool(name="psr", bufs=2, space="PSUM") as psr,
            tc.tile_pool(name="psb", bufs=1, space="PSUM") as psb,
        ):
            # prefetch the first two groups before anything else hits the
            # sync DMA queue
            xtiles = {}

            def load_group(g):
                pair = []
                for h in range(2):
                    t = xp.tile([P, NCOL], BF16, tag=f"xh{h}", name=f"xh{h}_{g}")
                    nc.sync.dma_start(out=t[:], in_=x0t_d[g * 2 + h])
                    pair.append(t)
                xtiles[g] = pair

            load_group(0)
            load_group(1)

            w_all = consts.tile([P, 16 * P], BF16, tag="w_all")
            nc.sync.dma_start(out=w_all[:], in_=wc_d[:])
            b_all = consts.tile([1, 4 * P], BF16, tag="b_all")
            nc.sync.dma_start(out=b_all[:], in_=bh_d[:])
            w0_sb = [w_all[:, i * P:(i + 1) * P] for i in range(8)]
            w1_sb = [w_all[:, (8 + i) * P:(9 + i) * P] for i in range(8)]
            b0_sb = [b_all[:, h * P:(h + 1) * P] for h in range(2)]
            b1_sb = [b_all[:, (2 + h) * P:(3 + h) * P] for h in range(2)]

            ident = consts.tile([P, P], F32, tag="ident")
            make_identity(nc, ident[:])
            ones1b = consts.tile([1, P], BF16, tag="ones1b")
            nc.vector.memset(ones1b[:], 1.0)
            ones1f = consts.tile([1, P], F32, tag="ones1f")
            nc.vector.memset(ones1f[:], 1.0)
            ones128b = consts.tile([P, 1], BF16, tag="ones128b")
            nc.vector.memset(ones128b[:], 1.0)
            ones128f = consts.tile([P, 1], F32, tag="ones128f")
            nc.vector.memset(ones128f[:], 1.0)
            epsb = consts.tile([1, 1], F32, tag="epsb")
            nc.vector.memset(epsb[:], 1e-30)
            epsp = consts.tile([P, 1], F32, tag="epsp")
            nc.vector.memset(epsp[:], 1e-30)
            # touch Sqrt early so its ACT table loads during startup
            warm = consts.tile([1, 1], F32, tag="warm")
            nc.scalar.activation(out=warm[:], in_=epsb[:], func=AF.Sqrt)

            h1_all = [
                consts.tile([P, NTOK], BF16, tag=f"h1_all{h}", name=f"h1_all{h}")
                for h in range(2)
            ]
            out_sb = consts.tile([P, D], F32, tag="out_sb")

            def agg_tree(xn, tag):
                # xn: [P, 25*P] bf16, 25 slot-major slabs; returns bf16 [P, P]
                # sum via tensor-tensor adds (fast-mode eligible on DVE).
                t1 = trp.tile([P, 12 * P], BF16, tag=f"t1{tag}")
                nc.vector.tensor_add(t1[:], xn[:, 0:12 * P], xn[:, 12 * P:24 * P])
                t2 = trp.tile([P, 6 * P], BF16, tag=f"t2{tag}")
                nc.vector.tensor_add(t2[:], t1[:, 0:6 * P], t1[:, 6 * P:12 * P])
                t3 = trp.tile([P, 3 * P], BF16, tag=f"t3{tag}")
                nc.vector.tensor_add(t3[:], t2[:, 0:3 * P], t2[:, 3 * P:6 * P])
                t4 = trp.tile([P, P], BF16, tag=f"t4{tag}")
                nc.vector.tensor_add(t4[:], t3[:, 0:P], t3[:, P:2 * P])
                t5 = trp.tile([P, P], BF16, tag=f"t5{tag}")
                nc.vector.tensor_add(t5[:], t4[:], t3[:, 2 * P:3 * P])
                ab = trp.tile([P, P], BF16, tag=f"ab{tag}")
                nc.vector.tensor_add(ab[:], t5[:], xn[:, 24 * P:25 * P])
                return ab

            def norm_scale(sqs, relu_src, dests, f32=False):
                # sqs: 2 sq half tiles; computes rinv per token and writes
                # dests[h] = relu_src[h] * rinv (column-wise scale).
                ones = ones128f if f32 else ones128b
                pn = psn.tile([P, 1], F32, tag="pn")
                nc.tensor.matmul(
                    out=pn[:], lhsT=sqs[0], rhs=ones[:], start=True, stop=False
                )
                nc.tensor.matmul(
                    out=pn[:], lhsT=sqs[1], rhs=ones[:], start=False, stop=True
                )
                ncol = epp.tile([P, 1], F32, tag="ncol")
                nc.scalar.activation(
                    out=ncol[:], in_=pn[:], func=AF.Sqrt, bias=epsp[:]
                )
                rcol = epp.tile([P, 1], F32, tag="rcol")
                nc.vector.reciprocal(out=rcol[:], in_=ncol[:])
                pr = psr.tile([1, P], F32, tag="pr")
                nc.tensor.matmul(
                    out=pr[:], lhsT=rcol[:], rhs=ident[:], start=True, stop=True
                )
                rrow = epp.tile([1, P], F32, tag="rrow")
                nc.scalar.copy(out=rrow[:], in_=pr[:])
                pb = psb.tile([P, P], F32, tag="pb")
                nc.tensor.matmul(
                    out=pb[:], lhsT=ones1f[:], rhs=rrow[:], start=True, stop=True
                )
                for h in range(2):
                    nc.vector.scalar_tensor_tensor(
                        out=dests[h], in0=relu_src[h], scalar=1.0, in1=pb[:],
                        op0=ALU.bypass, op1=ALU.mult,
                    )

            def mlp(x_chunks, w_sb, b_sb, hr_dests, edt):
                sqs = []
                for h in range(2):
                    ph = psh.tile([P, P], F32, tag="ph")
                    for k in range(4):
                        nc.tensor.matmul(
                            out=ph[:], lhsT=w_sb[k * 2 + h], rhs=x_chunks[k],
                            start=(k == 0), stop=False,
                        )
                    nc.tensor.matmul(
                        out=ph[:], lhsT=b_sb[h], rhs=ones1b[:],
                        start=False, stop=True,
                    )
                    nc.scalar.activation(out=hr_dests[h], in_=ph[:], func=AF.Relu)
                    sq = epp.tile([P, P], edt, tag=f"sq{h}")
                    nc.scalar.activation(out=sq[:], in_=hr_dests[h], func=AF.Square)
                    sqs.append(sq)
                return sqs

            # ---- layer 0: 11 groups of 128 tokens, streamed ----
            for g in range(NG):
                if g + 2 < NG:
                    load_group(g + 2)
                xh = xtiles.pop(g)
                aggb = [agg_tree(xh[h][:, P:], h) for h in range(2)]
                x_chunks = [xh[0][:, 0:P], xh[1][:, 0:P], aggb[0][:], aggb[1][:]]
                hr = [
                    epp.tile([P, P], BF16, tag=f"hr{h}", name=f"hr{h}_{g}")
                    for h in range(2)
                ]
                sqs = mlp(x_chunks, w0_sb, b0_sb, [hr[h][:] for h in range(2)], BF16)
                norm_scale(
                    [sqs[h][:] for h in range(2)],
                    [hr[h][:] for h in range(2)],
                    [h1_all[h][:, g * P:(g + 1) * P] for h in range(2)],
                )

            # ---- layer 1 ----
            agg1b = []
            for h in range(2):
                agg = trp.tile([P, P], F32, tag=f"agg1{h}")
                nc.vector.tensor_reduce(
                    out=agg[:],
                    in_=h1_all[h][:, P:].rearrange("p (t s) -> p t s", s=S1),
                    axis=AX.X, op=ALU.add,
                )
                ab = trp.tile([P, P], BF16, tag=f"agg1b{h}")
                nc.scalar.copy(out=ab[:], in_=agg[:])
                agg1b.append(ab)
            x_chunks = [
                h1_all[0][:, 0:P], h1_all[1][:, 0:P], agg1b[0][:], agg1b[1][:]
            ]
            hr2 = [
                epp.tile([P, P], F32, tag=f"hr2_{h}", name=f"hr2_{h}")
                for h in range(2)
            ]
            sqs2 = mlp(x_chunks, w1_sb, b1_sb, [hr2[h][:] for h in range(2)], F32)
            h2 = [
                epp.tile([P, P], F32, tag=f"h2_{h}", name=f"h2_{h}")
                for h in range(2)
            ]
            norm_scale(
                [sqs2[h][:] for h in range(2)],
                [hr2[h][:] for h in range(2)],
                [h2[h][:] for h in range(2)],
                f32=True,
            )
            # transpose h2T [d, t] -> out [t, d] and write
            for h in range(2):
                tp = psh.tile([P, P], F32, tag="ph")
                nc.tensor.transpose(out=tp[:], in_=h2[h][:], identity=ident[:])
                nc.scalar.copy(out=out_sb[:, h * P:(h + 1) * P], in_=tp[:])
            nc.sync.dma_start(out=out_d[:], in_=out_sb[:])

    nc.compile()
    return nc


def get_program():
    if "nc" not in _CACHE:
        _CACHE["nc"] = _build_program()
    return _CACHE["nc"]


def prepare_in_maps(features, W0, b0, W1, b1, nodes2, neigh2, neigh1):
    """Host-side sharding + index-resolved bf16 stream layout + weight prep."""
    import ml_dtypes

    bf16 = ml_dtypes.bfloat16
    feats16 = np.asarray(features, dtype=np.float32).astype(bf16)

    def chunk_w(W, fan):
        wt = np.ascontiguousarray(np.asarray(W, dtype=np.float32).T).copy()
        wt[D:, :] /= fan  # fold the neighbor mean into the weights
        # [k*2+h] = wt[k*128:(k+1)*128, h*128:(h+1)*128]
        return np.ascontiguousarray(
            wt.reshape(4, P, 2, P).transpose(0, 2, 1, 3).reshape(8, P, P)
        )

    w0c = chunk_w(W0, S0)
    w1c = chunk_w(W1, S1)
    # partition-major [P, 16P]: chunk i at columns [i*P, (i+1)*P)
    wc = np.concatenate([w0c, w1c], axis=0).transpose(1, 0, 2).reshape(P, 16 * P)
    wc = np.ascontiguousarray(wc).astype(bf16)
    bh = np.concatenate(
        [np.asarray(b0, np.float32), np.asarray(b1, np.float32)]
    ).reshape(1, 4 * P).astype(bf16)

    in_maps = []
    bc = B // N_CORES  # 128 targets per core
    for c in range(N_CORES):
        nodes2_c = nodes2[c * bc:(c + 1) * bc]
        neigh2_c = neigh2[c * bc:(c + 1) * bc, :]
        nodes1_c = np.concatenate([nodes2_c, neigh2_c.reshape(-1)])
        neigh1_c = np.concatenate(
            [
                neigh1[c * bc:(c + 1) * bc, :],
                neigh1[B + c * bc * S1:B + (c + 1) * bc * S1, :],
            ],
            axis=0,
        )
        # per group columns: [128 self | 25 slot-major slabs of 128 tokens]
        nodes1_g = nodes1_c.reshape(NG, P)
        neigh1_g = neigh1_c.reshape(NG, P, S0).transpose(0, 2, 1).reshape(NG, P * S0)
        flat = np.concatenate([nodes1_g, neigh1_g], axis=1).reshape(-1)
        rows = feats16[flat].reshape(NG, NCOL, D)
        x0t = np.ascontiguousarray(
            rows.transpose(0, 2, 1).reshape(NG, 2, P, NCOL)
        ).reshape(NG * 2, P, NCOL)
        in_maps.append({"x0t": x0t, "wc": wc, "bh": bh})
    return in_maps


def kernel(features, W0, b0, W1, b1, nodes2, neigh2, neigh1, _trace=False):
    from concourse.bass_utils import run_bass_kernel_spmd

    nc = get_program()
    in_maps = prepare_in_maps(features, W0, b0, W1, b1, nodes2, neigh2, neigh1)
    kwargs = {}
    if _trace:
        import tempfile

        import ntff_shim  # noqa: F401  (registers the axon NTFF hook)

        kwargs = {"trace": True, "tmpdir": tempfile.mkdtemp(prefix="ntff_")}
    res = run_bass_kernel_spmd(nc, in_maps, list(range(N_CORES)), **kwargs)
    out = np.concatenate([res.results[c]["out"] for c in range(N_CORES)], axis=0)
    if _trace:
        _CACHE["last_result"] = res
    return out
